# revision 1
# baseline (speedup 1.0000x reference)
"""Trainium2 Bass kernel for BitNet-style cross-attention (8 NeuronCores).

Strategy: pure data-parallel token sharding. b=2, n=2048 -> 4096 query-token
rows; each of the 8 cores owns 512 of them (cores 0-3 batch 0, 4-7 batch 1)
and computes its output slice fully independently (k/v for the core's batch
are recomputed per core -- on-chip collectives have ~10us/step latency floors,
far too slow at this kernel's ~250us scale).

All device tensors are feature-major ([dim, tokens]) so no on-chip transposes
are needed; the host supplies transposed views (pure layout transform).
Per-token absmax for activation quantization is computed with
gpsimd.partition_all_reduce(absmax), which conveniently replicates the result
across partitions. Quantized operands are dequantized to bf16 (integer values
up to 127 and ternary weights are exact in bf16; matmul accumulation in fp32
psum); softmax skips max-subtraction (logits are small by construction) and
obtains the denominator from a ones-column appended to the attn@v stationary
operand.
"""

import numpy as np

import concourse.bass as bass
import concourse.mybir as mybir
import concourse.tile as tile
from concourse import bacc, bass_isa
from concourse.bass_utils import run_bass_kernel_spmd

F32 = mybir.dt.float32
BF16 = mybir.dt.bfloat16
I8 = mybir.dt.int8
AX = mybir.AxisListType
OP = mybir.AluOpType
AF = mybir.ActivationFunctionType

P = 128

CFG_FULL = dict(DIM=1024, INNER=1024, H=16, D=64, NTOK=512, MCTX=2048)
N_CORES = 8
EPS = 1e-5


def build(cfg):
    DIM, INNER, H, D = cfg["DIM"], cfg["INNER"], cfg["H"], cfg["D"]
    NTOK, MCTX = cfg["NTOK"], cfg["MCTX"]
    KC = DIM // P          # input-dim chunks
    IC = INNER // P        # inner-dim chunks
    HPC = P // D           # heads per inner chunk (2)
    NKB = MCTX // P        # key blocks
    NTB = NTOK // P        # query-token 128-blocks
    CTB = MCTX // 512 if MCTX >= 512 else 1   # ctx 512-col blocks for k proj
    CW = min(512, MCTX)    # k-proj moving width
    NH = INNER // 512 if INNER >= 512 else 1  # inner 512-halves
    IW = min(512, INNER)
    SUB = min(256, NTOK)   # act-quant token sub-block
    VW = D + 1             # v columns per head incl ones

    nc = bacc.Bacc("TRN2", target_bir_lowering=False, debug=False,
                   num_devices=N_CORES)

    xT = nc.dram_tensor("xT", [DIM, NTOK], F32, kind="ExternalInput")
    cT = nc.dram_tensor("cT", [DIM, MCTX], F32, kind="ExternalInput")
    wT = {}
    for w in ("wq", "wk", "wv", "wo"):
        wT[w] = nc.dram_tensor(w + "T", [DIM, INNER], F32, kind="ExternalInput")
    y_out = nc.dram_tensor("y", [NTOK, DIM], F32, kind="ExternalOutput")

    from contextlib import ExitStack
    with tile.TileContext(nc) as tc, ExitStack() as ctx:
        pp = ctx.enter_context(tc.tile_pool(name="persist", bufs=1))
        smp = ctx.enter_context(tc.tile_pool(name="small", bufs=2))
        wsp = ctx.enter_context(tc.tile_pool(name="wstage", bufs=2))
        wbp = ctx.enter_context(tc.tile_pool(name="wbpool", bufs=1))
        ps_proj = ctx.enter_context(tc.tile_pool(name="ps_proj", bufs=2,
                                                 space="PSUM"))
        ps_sc = ctx.enter_context(tc.tile_pool(name="ps_sc", bufs=2,
                                               space="PSUM"))
        ps_o = ctx.enter_context(tc.tile_pool(name="ps_o", bufs=2,
                                              space="PSUM"))

        # ---- persistent SBUF tensors (live across phases) ----------------
        qb = pp.tile([P, IC * NTOK], BF16, tag="qb")      # q (scaled), T
        kb = pp.tile([P, IC * MCTX], BF16, tag="kb")      # k, T-major
        vb = pp.tile([P, NKB * H * VW], BF16, tag="vb")   # v natural + ones

        # ---- weight quantization -----------------------------------------
        wmean = {}

        def quant_weight(w):
            wpart = smp.tile([P, KC * (INNER // P)], F32, tag="wpart")
            for c in range(KC):
                s = wsp.tile([P, INNER], F32, tag="wst")
                nc.sync.dma_start(out=s[:], in_=wT[w].ap()[c * P:(c + 1) * P, :])
                nc.vector.tensor_reduce(
                    wpart[:, c * (INNER // P):(c + 1) * (INNER // P)],
                    s[:].rearrange("p (a b) -> p a b", b=P),
                    axis=AX.X, op=OP.add, apply_absolute_value=True)
            wsum = smp.tile([P, 1], F32, tag="wsum")
            nc.vector.tensor_reduce(wsum[:], wpart[:], axis=AX.X, op=OP.add)
            wrep = smp.tile([P, 1], F32, tag="wrep")
            nc.gpsimd.partition_all_reduce(wrep[:], wsum[:], channels=P,
                                           reduce_op=bass_isa.ReduceOp.add)
            mean = smp.tile([P, 1], F32, tag="wmean_" + w, name="mean_" + w)
            nc.vector.tensor_scalar(mean[:], wrep[:], 1.0 / (DIM * INNER),
                                    EPS, OP.mult, OP.max)
            qs = smp.tile([P, 1], F32, tag="wqs_" + w, name="qs_" + w)
            nc.vector.reciprocal(qs[:], mean[:])
            wmean[w] = mean
            wbt = wbp.tile([P, KC * INNER], BF16, tag="wb", name="wb_" + w)
            for c in range(KC):
                s = wsp.tile([P, INNER], F32, tag="wst2")
                nc.sync.dma_start(out=s[:], in_=wT[w].ap()[c * P:(c + 1) * P, :])
                nc.vector.tensor_scalar(s[:], s[:], qs[:], 1.49,
                                        OP.mult, OP.min)
                t8 = wsp.tile([P, INNER], I8, tag="wt8")
                nc.gpsimd.tensor_scalar(t8[:], s[:], -1.49, None, OP.max)
                nc.scalar.copy(
                    wbt[:, c * INNER:(c + 1) * INNER], t8[:])
            return wbt

        # ---- activation quantization (T-major) ---------------------------
        def act_quant(srcT, dstT, ncols, asp, s0, s1, dve_deq=True):
            for sblk in range(s0, s1):
                c0 = sblk * SUB
                stage = asp.tile([P, KC, SUB], F32, tag="astage")
                for c in range(KC):
                    nc.sync.dma_start(
                        out=stage[:, c, :],
                        in_=srcT.ap()[c * P:(c + 1) * P, c0:c0 + SUB])
                arep = asp.tile([P, KC, SUB], F32, tag="arep")
                nc.gpsimd.partition_all_reduce(
                    arep[:].rearrange("p a b -> p (a b)"),
                    stage[:].rearrange("p a b -> p (a b)"),
                    channels=P, reduce_op=bass_isa.ReduceOp.absmax)
                amax = asp.tile([P, SUB], F32, tag="amax")
                nc.vector.tensor_reduce(amax[:],
                                        arep[:].rearrange("p a b -> p b a"),
                                        axis=AX.X, op=OP.max)
                inv = asp.tile([P, SUB], F32, tag="ainv")
                nc.vector.tensor_scalar(inv[:], amax[:], EPS, 1.0 / 127.0,
                                        OP.max, OP.mult)
                qsc = asp.tile([P, SUB], F32, tag="aqsc")
                nc.vector.reciprocal(qsc[:], inv[:])
                for c in range(KC):
                    i8 = asp.tile([P, SUB], I8, tag="ai8")
                    nc.vector.tensor_tensor(i8[:], stage[:, c, :], qsc[:],
                                            op=OP.mult)
                    deq_eng = nc.vector if dve_deq else nc.gpsimd
                    deq_eng.tensor_tensor(
                        dstT[:, c * ncols + c0:c * ncols + c0 + SUB],
                        i8[:], inv[:], op=OP.mult)

        with ExitStack() as phase12:
            adp = phase12.enter_context(tc.tile_pool(name="adpool", bufs=1))
            asp = phase12.enter_context(tc.tile_pool(name="astage", bufs=2))
            xdT = adp.tile([P, KC * NTOK], BF16, tag="xdT")
            cdT = adp.tile([P, KC * MCTX], BF16, tag="cdT")

            # x quant, wq quant, then q projection starts the PE stream early
            act_quant(xT, xdT, NTOK, asp, 0, NTOK // SUB, dve_deq=True)
            wqb = quant_weight("wq")
            for ic in range(IC):
                ps = ps_proj.tile([P, NTOK], F32, tag="pp", name="psq")
                for c in range(KC):
                    nc.tensor.matmul(
                        ps[:],
                        wqb[:, c * INNER + ic * P: c * INNER + (ic + 1) * P],
                        xdT[:, c * NTOK:(c + 1) * NTOK],
                        start=(c == 0), stop=(c == KC - 1))
                nc.vector.tensor_copy(qb[:, ic * NTOK:(ic + 1) * NTOK], ps[:])

            wkb = quant_weight("wk")
            # scores scale folded into k eviction: mWq*mWk/sqrt(D)
            qkmul = smp.tile([P, 1], F32, tag="qkmul")
            nc.vector.tensor_tensor(qkmul[:], wmean["wq"][:], wmean["wk"][:],
                                    op=OP.mult)
            qksc = smp.tile([P, 1], F32, tag="qksc")
            nc.vector.tensor_scalar(qksc[:], qkmul[:], 1.0 / np.sqrt(D), None,
                                    OP.mult)
            # ctx quant interleaved with k projection per 512-col block
            for tb in range(CTB):
                act_quant(cT, cdT, MCTX, asp,
                          tb * (CW // SUB), (tb + 1) * (CW // SUB),
                          dve_deq=True)
                for ic in range(IC):
                    ps = ps_proj.tile([P, CW], F32, tag="pp", name="psk")
                    for c in range(KC):
                        nc.tensor.matmul(
                            ps[:],
                            wkb[:, c * INNER + ic * P: c * INNER + (ic + 1) * P],
                            cdT[:, c * MCTX + tb * CW: c * MCTX + (tb + 1) * CW],
                            start=(c == 0), stop=(c == KC - 1))
                    nc.scalar.mul(
                        kb[:, ic * MCTX + tb * CW: ic * MCTX + (tb + 1) * CW],
                        ps[:], qksc[:])
            wvb = quant_weight("wv")
            vb3 = vb[:].rearrange("p (k h w) -> p k h w", h=H, w=VW)
            nc.vector.memset(vb3[:, :, :, D], 1.0)
            for kbk in range(NKB):
                    for ih in range(NH):
                        ps = ps_proj.tile([P, IW], F32, tag="pp", name="psv")
                        for c in range(KC):
                            nc.tensor.matmul(
                                ps[:],
                                cdT[:, c * MCTX + kbk * P: c * MCTX + (kbk + 1) * P],
                                wvb[:, c * INNER + ih * IW: c * INNER + (ih + 1) * IW],
                                start=(c == 0), stop=(c == KC - 1))
                        hph = IW // D
                        nc.scalar.mul(
                            vb3[:, kbk, ih * hph:(ih + 1) * hph, 0:D],
                            ps[:].rearrange("p (h d) -> p h d", d=D),
                            wmean["wv"][:])

        # ---- attention ---------------------------------------------------
        op_pool = ctx.enter_context(tc.tile_pool(name="opool", bufs=1))
        otT = op_pool.tile([P, IC * NTOK], F32, tag="otT")
        oqdT = op_pool.tile([P, IC * NTOK], BF16, tag="oqdT")
        with tc.tile_pool(name="etile", bufs=4) as ep:
            for hp in range(H // 2):
                hA, hB = 2 * hp, 2 * hp + 1
                icA, pA = (hA * D) // P, (hA * D) % P
                icB, pB = (hB * D) // P, (hB * D) % P
                popool, potag = (ps_o, "po") if hp % 2 == 0 else (ps_proj, "pp")
                po = [popool.tile([P, NTOK], F32, tag=potag, name=f"po{hp}_{j}")
                      for j in range(2)]
                for kbk in range(NKB):
                    ss = ps_sc.tile([P, 2, NTOK], F32, tag="ss", name="ss")
                    for j, (h, ich, ph) in enumerate(
                            [(hA, icA, pA), (hB, icB, pB)]):
                        nc.tensor.matmul(
                            ss[:, j, :],
                            kb[ph:ph + D,
                               ich * MCTX + kbk * P: ich * MCTX + (kbk + 1) * P],
                            qb[ph:ph + D, ich * NTOK:(ich + 1) * NTOK],
                            start=True, stop=True)
                    et = ep.tile([P, 2, NTOK], BF16, tag="et")
                    nc.scalar.activation(et[:], ss[:], AF.Exp)
                    for j, h in enumerate((hA, hB)):
                        nc.tensor.matmul(
                            po[j][0:VW, :],
                            vb3[:, kbk, h, :],
                            et[:, j, :],
                            start=(kbk == 0), stop=(kbk == NKB - 1))
                for j, (h, ich, ph) in enumerate([(hA, icA, pA), (hB, icB, pB)]):
                    rd = smp.tile([1, NTOK], F32, tag="rd")
                    nc.vector.reciprocal(rd[:], po[j][D:D + 1, :])
                    rb = smp.tile([D, NTOK], F32, tag="rb")
                    nc.gpsimd.partition_broadcast(rb[:], rd[:])
                    nc.vector.tensor_tensor(
                        otT[ph:ph + D, ich * NTOK:(ich + 1) * NTOK],
                        po[j][0:D, :], rb[:], op=OP.mult)

        # ---- attn-out quantization + output projection -------------------
        with tc.tile_pool(name="oq", bufs=2) as oqp, \
                tc.tile_pool(name="ysb", bufs=2) as yp:
            wob = quant_weight("wo")
            ot3 = otT[:].rearrange("p (c t) -> p c t", c=IC)
            oamax = smp.tile([P, NTOK], F32, tag="oamax")
            for c in range(IC):
                arep = oqp.tile([P, NTOK], F32, tag="oarep")
                nc.gpsimd.partition_all_reduce(
                    arep[:], ot3[:, c, :], channels=P,
                    reduce_op=bass_isa.ReduceOp.absmax)
                if c == 0:
                    nc.vector.tensor_copy(oamax[:], arep[:])
                else:
                    nc.vector.tensor_tensor(oamax[:], oamax[:], arep[:],
                                            op=OP.max)
            oinv = smp.tile([P, NTOK], F32, tag="oinv")
            nc.vector.tensor_scalar(oinv[:], oamax[:], EPS, 1.0 / 127.0,
                                    OP.max, OP.mult)
            oqsc = smp.tile([P, NTOK], F32, tag="oqsc")
            nc.vector.reciprocal(oqsc[:], oinv[:])
            for c in range(IC):
                i8 = oqp.tile([P, NTOK], I8, tag="oi8")
                nc.vector.tensor_tensor(i8[:], ot3[:, c, :], oqsc[:], op=OP.mult)
                nc.gpsimd.tensor_tensor(oqdT[:, c * NTOK:(c + 1) * NTOK],
                                        i8[:], oinv[:], op=OP.mult)

            for tb in range(NTB):
                for oh in range(DIM // IW):
                    ps = ps_proj.tile([P, IW], F32, tag="pp", name="psy")
                    for c in range(IC):
                        nc.tensor.matmul(
                            ps[:],
                            oqdT[:, c * NTOK + tb * P: c * NTOK + (tb + 1) * P],
                            wob[:, c * INNER + oh * IW: c * INNER + (oh + 1) * IW],
                            start=(c == 0), stop=(c == IC - 1))
                    ysb = yp.tile([P, IW], F32, tag="ysb")
                    nc.scalar.mul(ysb[:], ps[:], wmean["wo"][:])
                    nc.sync.dma_start(
                        out=y_out.ap()[tb * P:(tb + 1) * P,
                                       oh * IW:(oh + 1) * IW],
                        in_=ysb[:])
    nc.compile()
    return nc


_CACHE = {}


def _get_nc(key, cfg):
    if key not in _CACHE:
        _CACHE[key] = build(cfg)
    return _CACHE[key]


def _shard(x, context, wq, wk, wv, wo, NTOK):
    b = x.shape[0]
    wmaps = {w + "T": np.ascontiguousarray(a.T)
             for w, a in (("wq", wq), ("wk", wk), ("wv", wv), ("wo", wo))}
    cores_per_b = N_CORES // b
    in_maps = []
    for core in range(N_CORES):
        bi = core // cores_per_b
        t0 = (core % cores_per_b) * NTOK
        in_maps.append(dict(
            xT=np.ascontiguousarray(x[bi, t0:t0 + NTOK, :].T),
            cT=np.ascontiguousarray(context[bi].T),
            **wmaps))
    return in_maps


def _assemble(results, b, n, dim, NTOK):
    out = np.empty((b, n, dim), dtype=np.float32)
    cores_per_b = N_CORES // b
    for core in range(N_CORES):
        bi = core // cores_per_b
        t0 = (core % cores_per_b) * NTOK
        out[bi, t0:t0 + NTOK, :] = results[core]["y"]
    return out


def run(x, context, wq, wk, wv, wo, trace=False):
    cfg = CFG_FULL
    b, n, dim = x.shape
    NTOK = cfg["NTOK"]
    nc = _get_nc("full", cfg)
    in_maps = _shard(x, context, wq, wk, wv, wo, NTOK)
    res = run_bass_kernel_spmd(nc, in_maps, list(range(N_CORES)), trace=trace)
    return _assemble(res.results, b, n, dim, NTOK), res


def kernel(x, context, wq, wk, wv, wo):
    return run(x, context, wq, wk, wv, wo, trace=False)[0]


if __name__ == "__main__":
    ins = {k: np.random.randn(*s).astype(np.float32) * (0.02 if k[0] == 'w' else 1.0)
           for k, s in [("x", (2, 2048, 1024)), ("context", (2, 2048, 1024)),
                        ("wq", (1024, 1024)), ("wk", (1024, 1024)),
                        ("wv", (1024, 1024)), ("wo", (1024, 1024))]}
    y = kernel(**ins)
    print("kernel output", y.shape, y.dtype, np.abs(y).max())



# revision 7
# speedup vs baseline: 1.7729x; 1.7729x over previous
"""Trainium2 Bass kernel for BitNet-style cross-attention (8 NeuronCores).

Data-parallel token sharding: b=2, n=2048 -> 4096 query-token rows; each of
the 8 cores owns 512 (cores 0-3 batch 0, 4-7 batch 1) and computes its output
slice independently (k/v recomputed per core).

Key ideas vs the naive formulation:
 - Per-token activation-quant scales factor out of every GEMM, so activations
   are matmul'd as raw int8-valued bf16 (ints <= 127 are exact in bf16) and
   all dequant scales are folded into PSUM evictions / the softmax exp:
     * q eviction is multiplied by inv_x[t] (per-column replicated tile, DVE),
     * k eviction is a plain PSUM->SBUF copy (Act engine),
     * v eviction and the exp() get per-ctx-token scales as per-partition
       Act-engine scale operands (icT, obtained via tiny PE transposes of the
       replicated inv_c rows); exp scale also carries mq*mk/sqrt(D),
     * the attention output stays UNNORMALIZED; the softmax denominator
       cancels inside the output act-quant and is applied during the
       normalize step only.
 - round() is implemented with the fp32 magic-constant trick
   ((v + 1.5*2^23) - 1.5*2^23 == round-half-even) in fused DVE tensor_scalar
   ops -- no int8 round-trips and no GpSimd casts (GpSimd f32->i8 measured
   ~6% efficiency and starves the DVE via SBUF contention).
 - Weight quant: Act does Abs+accum (absmean) and the final Copy(+bias=-M)
   to ternary bf16; DVE does two fused tensor_scalar ops. All in fp32, exact
   same rounding as the jax reference.
"""

import numpy as np

import concourse.bass as bass
import concourse.mybir as mybir
import concourse.tile as tile
from concourse import bacc, bass_isa
from concourse.bass_utils import run_bass_kernel_spmd

F32 = mybir.dt.float32
BF16 = mybir.dt.bfloat16
AX = mybir.AxisListType
OP = mybir.AluOpType
AF = mybir.ActivationFunctionType

P = 128
MAGIC = 12582912.0  # 1.5 * 2**23: fp32 add/sub rounds to nearest int (ties even)

CFG_FULL = dict(DIM=1024, INNER=1024, H=16, D=64, NTOK=512, MCTX=2048)
N_CORES = 8
EPS = 1e-5


def build(cfg):
    DIM, INNER, H, D = cfg["DIM"], cfg["INNER"], cfg["H"], cfg["D"]
    NTOK, MCTX = cfg["NTOK"], cfg["MCTX"]
    KC = DIM // P            # input-dim 128-chunks (8)
    IC = INNER // P          # inner-dim 128-chunks (8)
    NKB = MCTX // P          # ctx 128-blocks (16)
    NTB = NTOK // P          # query-token 128-blocks (4)
    NQ = 4                   # ctx quarters
    QTOK = MCTX // NQ        # 512 ctx tokens per quarter
    KBQ = QTOK // P          # ctx 128-blocks per quarter (4)
    VW = D + 1               # v columns per head incl ones

    nc = bacc.Bacc("TRN2", target_bir_lowering=False, debug=False,
                   num_devices=N_CORES)

    xT = nc.dram_tensor("xT", [DIM, NTOK], F32, kind="ExternalInput")
    cT = nc.dram_tensor("cT", [DIM, MCTX], F32, kind="ExternalInput")
    wT = {}
    for w in ("wq", "wk", "wv", "wo"):
        wT[w] = nc.dram_tensor(w + "T", [DIM, INNER], F32, kind="ExternalInput")
    iden = nc.dram_tensor("iden", [P, P], F32, kind="ExternalInput")
    y_out = nc.dram_tensor("y", [NTOK, DIM], F32, kind="ExternalOutput")

    from contextlib import ExitStack
    with tile.TileContext(nc) as tc, ExitStack() as ctx:
        # ---- persistent pools -------------------------------------------
        pp = ctx.enter_context(tc.tile_pool(name="persist", bufs=1))
        smp = ctx.enter_context(tc.tile_pool(name="small", bufs=2))

        qb = pp.tile([P, IC, NTOK], BF16, tag="qb")     # q*inv_x, feat-major
        kb = pp.tile([P, IC, MCTX], BF16, tag="kb")     # k raw ints, feat-major
        vb = pp.tile([P, NKB * H * VW], BF16, tag="vb")  # v natural + ones col
        vb3 = vb[:].rearrange("p (k h w) -> p k h w", h=H, w=VW)
        idt = pp.tile([P, P], F32, tag="idt")           # identity for PE transp
        nc.sync.dma_start(out=idt[:], in_=iden.ap()[:, :])
        icT = pp.tile([P, NKB], F32, tag="icT")         # inv_c, ctx-token-major
        vsc = pp.tile([P, NKB], F32, tag="vsc")         # icT * mean|wv|
        esc = pp.tile([P, NKB], F32, tag="esc")         # icT * mq*mk/sqrt(D)

        wmean = {}

        # ---- weight quantization ----------------------------------------
        # ws (fp32 staging) -> Act Abs+accum -> gps allreduce -> qs=1/mean
        # DVE: t=min(w*qs,1.49); t=max(t,-1.49)+MAGIC ; Act: copy(t-MAGIC)->bf16
        def quant_weight(w, wsp, dst_pool):
            ws = wsp.tile([P, KC * INNER], F32, tag="wst")
            for c in range(KC):
                nc.sync.dma_start(out=ws[:, c * INNER:(c + 1) * INNER],
                                  in_=wT[w].ap()[c * P:(c + 1) * P, :])
            wbt = dst_pool.tile([P, KC * INNER], BF16, tag="wb_" + w,
                                name="wb_" + w)
            wsum = smp.tile([P, 1], F32, tag="wsum")
            nc.scalar.activation(wbt[:], ws[:], AF.Abs, accum_out=wsum[:])
            wrep = smp.tile([P, 1], F32, tag="wrep")
            nc.gpsimd.partition_all_reduce(wrep[:], wsum[:], channels=P,
                                           reduce_op=bass_isa.ReduceOp.add)
            mean = smp.tile([P, 1], F32, tag="wmean_" + w, name="mean_" + w)
            nc.vector.tensor_scalar(mean[:], wrep[:], 1.0 / (DIM * INNER),
                                    EPS, OP.mult, OP.max)
            qs = smp.tile([P, 1], F32, tag="wqs_" + w, name="qs_" + w)
            nc.vector.reciprocal(qs[:], mean[:])
            wmean[w] = mean
            nc.vector.tensor_scalar(ws[:], ws[:], qs[:], 1.49, OP.mult, OP.min)
            nc.vector.tensor_scalar(ws[:], ws[:], -1.49, MAGIC, OP.max, OP.add)
            nc.scalar.activation(wbt[:], ws[:], AF.Copy, bias=-MAGIC)
            return wbt

        # ---- activation quantization (feature-major, no dequant) --------
        # src fp32 [P, KC, ncol]; writes int-valued bf16 into dst[:, c, col0:]
        # and the replicated dequant scale inv_rep[:, 0:ncol].
        def act_quant(asp, src, dst, dcol0, inv_rep, ncol):
            amax = asp.tile([P, ncol], F32, tag="amax")
            nc.vector.tensor_reduce(
                amax[:], src.rearrange("p c t -> p t c"),
                axis=AX.X, op=OP.max, apply_absolute_value=True)
            rep = asp.tile([P, ncol], F32, tag="arep")
            nc.gpsimd.partition_all_reduce(rep[:], amax[:], channels=P,
                                           reduce_op=bass_isa.ReduceOp.max)
            nc.vector.tensor_scalar(inv_rep, rep[:], EPS, 1.0 / 127.0,
                                    OP.max, OP.mult)
            nc.vector.tensor_scalar(rep[:], rep[:], EPS, None, OP.max)
            rq = asp.tile([P, ncol], F32, tag="arq")
            nc.vector.reciprocal(rq[:], rep[:])
            for c in range(KC):
                tmp = asp.tile([P, ncol], F32, tag="atmp")
                nc.vector.scalar_tensor_tensor(
                    tmp[:], src[:, c, :], 127.0, rq[:], op0=OP.mult,
                    op1=OP.mult)
                nc.vector.tensor_scalar(
                    dst[:, c, dcol0:dcol0 + ncol], tmp[:], MAGIC, -MAGIC,
                    OP.add, OP.add)

        with ExitStack() as ph1:
            asp = ph1.enter_context(tc.tile_pool(name="astage", bufs=2))
            xqp = ph1.enter_context(tc.tile_pool(name="xq", bufs=1))
            ps_a = ph1.enter_context(tc.tile_pool(name="ps_a", bufs=3,
                                                  space="PSUM"))
            ps_v = ph1.enter_context(tc.tile_pool(name="ps_v", bufs=2,
                                                  space="PSUM"))
            ps_t = ph1.enter_context(tc.tile_pool(name="ps_t", bufs=1,
                                                  space="PSUM"))

            xdq = xqp.tile([P, KC, NTOK], BF16, tag="xdq")
            inv_x = xqp.tile([P, NTOK], F32, tag="invx")
            with tc.tile_pool(name="xstage", bufs=1) as xsp:
                xs = xsp.tile([P, KC, NTOK], F32, tag="xs")
                for c in range(KC):
                    nc.sync.dma_start(out=xs[:, c, :],
                                      in_=xT.ap()[c * P:(c + 1) * P, :])
                act_quant(asp, xs[:], xdq[:], 0, inv_x[:], NTOK)

            with ExitStack() as wph:
                wbp = wph.enter_context(tc.tile_pool(name="wbq", bufs=1))
                with tc.tile_pool(name="wstage", bufs=1) as wsp:
                    wqb = quant_weight("wq", wsp, wbp)

                    # Q projection (starts PE early; eviction scale inv_x)
                    wqb3 = wqb[:].rearrange("p (c i) -> p c i", c=KC)
                    for ic in range(IC):
                        pq = ps_a.tile([P, NTOK], F32, tag="psa",
                                       name=f"pq{ic}")
                        for c in range(KC):
                            nc.tensor.matmul(
                                pq[:], wqb3[:, c, ic * P:(ic + 1) * P],
                                xdq[:, c, :],
                                start=(c == 0), stop=(c == KC - 1))
                        nc.vector.tensor_tensor(qb[:, ic, :], pq[:],
                                                inv_x[:], op=OP.mult)

                    wkb = quant_weight("wk", wsp, wbp)
                    wvb = quant_weight("wv", wsp, wbp)
                wkb3 = wkb[:].rearrange("p (c i) -> p c i", c=KC)
                wvb3 = wvb[:].rearrange("p (c i) -> p c i", c=KC)

                # exp scale constant: mq * mk / sqrt(D)
                qkm = smp.tile([P, 1], F32, tag="qkm")
                nc.vector.tensor_tensor(qkm[:], wmean["wq"][:],
                                        wmean["wk"][:], op=OP.mult)
                nc.vector.tensor_scalar(qkm[:], qkm[:],
                                        1.0 / float(np.sqrt(D)), None,
                                        OP.mult)

                # ones column of v (evictions only touch [0:D])
                nc.vector.memset(vb3[:, :, :, D], 1.0)

                # ctx quarters: load -> quant -> icT -> K-proj -> V-proj
                with tc.tile_pool(name="cstage", bufs=2) as csp:
                    for q in range(NQ):
                        col0 = q * QTOK
                        cs = csp.tile([P, KC, QTOK], F32, tag="cs",
                                      name=f"cs{q}")
                        for c in range(KC):
                            nc.sync.dma_start(
                                out=cs[:, c, :],
                                in_=cT.ap()[c * P:(c + 1) * P,
                                            col0:col0 + QTOK])
                        inv_c = csp.tile([P, QTOK], F32, tag="invc",
                                         name=f"invc{q}")
                        cdq = csp.tile([P, KC, QTOK], BF16, tag="cdq",
                                       name=f"cdq{q}")
                        act_quant(asp, cs[:], cdq[:], 0, inv_c[:], QTOK)

                        # icT: per-ctx-token inv_c via PE transposes
                        for kk in range(KBQ):
                            kbk = q * KBQ + kk
                            pt = ps_t.tile([P, P], F32, tag="pt",
                                           name=f"pt{kbk}")
                            nc.tensor.transpose(
                                pt[:], inv_c[:, kk * P:(kk + 1) * P], idt[:])
                            nc.scalar.copy(icT[:, kbk:kbk + 1], pt[:, 0:1])
                        sl = slice(q * KBQ, (q + 1) * KBQ)
                        nc.vector.tensor_scalar(vsc[:, sl], icT[:, sl],
                                                wmean["wv"][:], None, OP.mult)
                        nc.vector.tensor_scalar(esc[:, sl], icT[:, sl],
                                                qkm[:], None, OP.mult)

                        # K projection for this quarter
                        for ic in range(IC):
                            pk = ps_a.tile([P, QTOK], F32, tag="psa",
                                           name=f"pk{q}_{ic}")
                            for c in range(KC):
                                nc.tensor.matmul(
                                    pk[:], wkb3[:, c, ic * P:(ic + 1) * P],
                                    cdq[:, c, :],
                                    start=(c == 0), stop=(c == KC - 1))
                            nc.scalar.copy(kb[:, ic, col0:col0 + QTOK], pk[:])

                        # V projection for this quarter
                        for kk in range(KBQ):
                            kbk = q * KBQ + kk
                            pv = ps_v.tile([P, 2, INNER // 2], F32, tag="psv",
                                           name=f"pv{kbk}")
                            for c in range(KC):
                                for ih in range(2):
                                    nc.tensor.matmul(
                                        pv[:, ih, :],
                                        cdq[:, c, kk * P:(kk + 1) * P],
                                        wvb3[:, c, ih * (INNER // 2):
                                             (ih + 1) * (INNER // 2)],
                                        start=(c == 0), stop=(c == KC - 1))
                            hph = (INNER // 2) // D  # heads per half (8)
                            for ih in range(2):
                                nc.scalar.mul(
                                    vb3[:, kbk, ih * hph:(ih + 1) * hph, 0:D],
                                    pv[:, ih, :].rearrange(
                                        "p (h d) -> p h d", d=D),
                                    vsc[:, kbk:kbk + 1])

        # ---- attention ---------------------------------------------------
        wop = ctx.enter_context(tc.tile_pool(name="wopool", bufs=1))
        op_pool = ctx.enter_context(tc.tile_pool(name="opool", bufs=1))
        otT = op_pool.tile([P, IC, NTOK], F32, tag="otT")
        with ExitStack() as ph2:
            ep = ph2.enter_context(tc.tile_pool(name="etile", bufs=4))
            rbp = ph2.enter_context(tc.tile_pool(name="rbpool", bufs=2))
            ps_sc = ph2.enter_context(tc.tile_pool(name="ps_sc", bufs=2,
                                                   space="PSUM"))
            ps_o = ph2.enter_context(tc.tile_pool(name="ps_o", bufs=2,
                                                  space="PSUM"))
            ps_o2 = ph2.enter_context(tc.tile_pool(name="ps_o2", bufs=2,
                                                   space="PSUM"))
            wsp2 = ph2.enter_context(tc.tile_pool(name="wstage2", bufs=1))

            wob = None
            for hp in range(H // 2):
                hA, hB = 2 * hp, 2 * hp + 1
                pA, pB = (hA * D) % P, (hB * D) % P
                popool, potag = (ps_o, "po") if hp % 2 == 0 else (ps_o2, "po2")
                po = [popool.tile([VW, NTOK], F32, tag=potag,
                                  name=f"po{hp}_{j}") for j in range(2)]
                for kbk in range(NKB):
                    ss = ps_sc.tile([P, 2, NTOK], F32, tag="ss", name="ss")
                    for j, (h, ph) in enumerate([(hA, pA), (hB, pB)]):
                        nc.tensor.matmul(
                            ss[:, j, :],
                            kb[ph:ph + D, hp, kbk * P:(kbk + 1) * P],
                            qb[ph:ph + D, hp, :],
                            start=True, stop=True)
                    et = ep.tile([P, 2, NTOK], BF16, tag="et")
                    nc.scalar.activation(et[:], ss[:], AF.Exp,
                                         scale=esc[:, kbk:kbk + 1])
                    for j, h in enumerate((hA, hB)):
                        nc.tensor.matmul(
                            po[j][0:VW, :],
                            vb3[:, kbk, h, :],
                            et[:, j, :],
                            start=(kbk == 0), stop=(kbk == NKB - 1))
                # normalize: ot = po[0:D] / den  (den = po[D] row)
                for j, ph in enumerate((pA, pB)):
                    rd = rbp.tile([1, NTOK], F32, tag="rd")
                    nc.vector.reciprocal(rd[:], po[j][D:D + 1, :])
                    rb = rbp.tile([D, NTOK], F32, tag="rb")
                    nc.gpsimd.partition_broadcast(rb[:], rd[:])
                    nc.vector.tensor_tensor(
                        otT[ph:ph + D, hp, :], po[j][0:D, :], rb[:],
                        op=OP.mult)
                if hp == 0:
                    # wo quant rides the attention phase (engines are free)
                    wob = quant_weight("wo", wsp2, wop)

        # ---- attn-out quantization + output projection ------------------
        with tc.tile_pool(name="oq", bufs=2) as oqp, \
                tc.tile_pool(name="ysb", bufs=2) as yp, \
                tc.tile_pool(name="ps_y", bufs=2, space="PSUM") as ps_y, \
                tc.tile_pool(name="ps_t2", bufs=1, space="PSUM") as ps_t2:
            odq = op_pool.tile([P, IC, NTOK], BF16, tag="odq")
            inv_o = op_pool.tile([P, NTOK], F32, tag="invo")
            act_quant(oqp, otT[:], odq[:], 0, inv_o[:], NTOK)

            # y-eviction scale, token-major: syT = (inv_o).T * mean|wo|
            syT = smp.tile([P, NTB], F32, tag="syT")
            for tb in range(NTB):
                pt = ps_t2.tile([P, P], F32, tag="pt2", name=f"pt2{tb}")
                nc.tensor.transpose(pt[:], inv_o[:, tb * P:(tb + 1) * P],
                                    idt[:])
                nc.scalar.copy(syT[:, tb:tb + 1], pt[:, 0:1])
            nc.vector.tensor_scalar(syT[:], syT[:], wmean["wo"][:], None,
                                    OP.mult)

            wob3 = wob[:].rearrange("p (c i) -> p c i", c=IC)
            for tb in range(NTB):
                py = ps_y.tile([P, 2, DIM // 2], F32, tag="psy",
                               name=f"py{tb}")
                for c in range(IC):
                    for oh in range(2):
                        nc.tensor.matmul(
                            py[:, oh, :],
                            odq[:, c, tb * P:(tb + 1) * P],
                            wob3[:, c, oh * (DIM // 2):(oh + 1) * (DIM // 2)],
                            start=(c == 0), stop=(c == IC - 1))
                ysb = yp.tile([P, DIM], F32, tag="ysb")
                nc.scalar.mul(ysb[:], py[:].rearrange("p a b -> p (a b)"),
                              syT[:, tb:tb + 1])
                nc.sync.dma_start(
                    out=y_out.ap()[tb * P:(tb + 1) * P, :], in_=ysb[:])
    nc.compile()
    return nc


_CACHE = {}


def _get_nc(key, cfg):
    if key not in _CACHE:
        _CACHE[key] = build(cfg)
    return _CACHE[key]


def _shard(x, context, wq, wk, wv, wo, NTOK):
    b = x.shape[0]
    wmaps = {w + "T": np.ascontiguousarray(a.T)
             for w, a in (("wq", wq), ("wk", wk), ("wv", wv), ("wo", wo))}
    wmaps["iden"] = np.eye(128, dtype=np.float32)
    cores_per_b = N_CORES // b
    in_maps = []
    for core in range(N_CORES):
        bi = core // cores_per_b
        t0 = (core % cores_per_b) * NTOK
        in_maps.append(dict(
            xT=np.ascontiguousarray(x[bi, t0:t0 + NTOK, :].T),
            cT=np.ascontiguousarray(context[bi].T),
            **wmaps))
    return in_maps


def _assemble(results, b, n, dim, NTOK):
    out = np.empty((b, n, dim), dtype=np.float32)
    cores_per_b = N_CORES // b
    for core in range(N_CORES):
        bi = core // cores_per_b
        t0 = (core % cores_per_b) * NTOK
        out[bi, t0:t0 + NTOK, :] = results[core]["y"]
    return out


def run(x, context, wq, wk, wv, wo, trace=False):
    cfg = CFG_FULL
    b, n, dim = x.shape
    NTOK = cfg["NTOK"]
    nc = _get_nc("full", cfg)
    in_maps = _shard(x, context, wq, wk, wv, wo, NTOK)
    res = run_bass_kernel_spmd(nc, in_maps, list(range(N_CORES)), trace=trace)
    return _assemble(res.results, b, n, dim, NTOK), res


def kernel(x, context, wq, wk, wv, wo):
    return run(x, context, wq, wk, wv, wo, trace=False)[0]


if __name__ == "__main__":
    ins = {k: np.random.randn(*s).astype(np.float32) * (0.02 if k[0] == 'w' else 1.0)
           for k, s in [("x", (2, 2048, 1024)), ("context", (2, 2048, 1024)),
                        ("wq", (1024, 1024)), ("wk", (1024, 1024)),
                        ("wv", (1024, 1024)), ("wo", (1024, 1024))]}
    y = kernel(**ins)
    print("kernel output", y.shape, y.dtype, np.abs(y).max())


# revision 12
# speedup vs baseline: 2.0033x; 1.1299x over previous
"""Trainium2 Bass kernel for BitNet-style cross-attention (8 NeuronCores).

Data-parallel token sharding: b=2, n=2048 -> 4096 query-token rows; each of
the 8 cores owns 512 (cores 0-3 batch 0, 4-7 batch 1) and computes its output
slice independently (k/v recomputed per core).

Key ideas vs the naive formulation:
 - Per-token activation-quant scales factor out of every GEMM, so activations
   are matmul'd as raw int8-valued bf16 (ints <= 127 are exact in bf16) and
   all dequant scales are folded into PSUM evictions / the softmax exp:
     * q eviction is multiplied by inv_x[t] (per-column replicated tile, DVE),
     * k eviction is a plain PSUM->SBUF copy (Act engine),
     * v eviction and the exp() get per-ctx-token scales as per-partition
       Act-engine scale operands (icT, obtained via tiny PE transposes of the
       replicated inv_c rows); exp scale also carries mq*mk/sqrt(D),
     * the attention output stays UNNORMALIZED; the softmax denominator
       cancels inside the output act-quant and is applied during the
       normalize step only.
 - round() is implemented with the fp32 magic-constant trick
   ((v + 1.5*2^23) - 1.5*2^23 == round-half-even) in fused DVE tensor_scalar
   ops -- no int8 round-trips and no GpSimd casts (GpSimd f32->i8 measured
   ~6% efficiency and starves the DVE via SBUF contention).
 - Weight quant: Act does Abs+accum (absmean) and the final Copy(+bias=-M)
   to ternary bf16; DVE does two fused tensor_scalar ops. All in fp32, exact
   same rounding as the jax reference.
"""

import numpy as np

import concourse.bass as bass
import concourse.mybir as mybir
import concourse.tile as tile
from concourse import bacc, bass_isa
from concourse.bass_utils import run_bass_kernel_spmd

F32 = mybir.dt.float32
BF16 = mybir.dt.bfloat16
AX = mybir.AxisListType
OP = mybir.AluOpType
AF = mybir.ActivationFunctionType

P = 128
MAGIC = 12582912.0  # 1.5 * 2**23: fp32 add/sub rounds to nearest int (ties even)

CFG_FULL = dict(DIM=1024, INNER=1024, H=16, D=64, NTOK=512, MCTX=2048)
N_CORES = 8
EPS = 1e-5


def build(cfg):
    DIM, INNER, H, D = cfg["DIM"], cfg["INNER"], cfg["H"], cfg["D"]
    NTOK, MCTX = cfg["NTOK"], cfg["MCTX"]
    KC = DIM // P            # input-dim 128-chunks (8)
    IC = INNER // P          # inner-dim 128-chunks (8)
    NKB = MCTX // P          # ctx 128-blocks (16)
    NTB = NTOK // P          # query-token 128-blocks (4)
    NQ = 4                   # ctx quarters
    QTOK = MCTX // NQ        # 512 ctx tokens per quarter
    KBQ = QTOK // P          # ctx 128-blocks per quarter (4)
    VW = D + 1               # v columns per head incl ones

    nc = bacc.Bacc("TRN2", target_bir_lowering=False, debug=False,
                   num_devices=N_CORES)

    xT = nc.dram_tensor("xT", [DIM, NTOK], F32, kind="ExternalInput")
    cT = nc.dram_tensor("cT", [DIM, MCTX], F32, kind="ExternalInput")
    wT = {}
    for w in ("wq", "wk", "wv", "wo"):
        wT[w] = nc.dram_tensor(w + "T", [DIM, INNER], F32, kind="ExternalInput")
    iden = nc.dram_tensor("iden", [P, P], F32, kind="ExternalInput")
    y_out = nc.dram_tensor("y", [NTOK, DIM], F32, kind="ExternalOutput")

    from contextlib import ExitStack
    with tile.TileContext(nc) as tc, ExitStack() as ctx:
        # ---- persistent pools -------------------------------------------
        pp = ctx.enter_context(tc.tile_pool(name="persist", bufs=1))
        smp = ctx.enter_context(tc.tile_pool(name="small", bufs=2))

        qb = pp.tile([P, IC, NTOK], BF16, tag="qb")     # q*inv_x, feat-major
        kb = pp.tile([P, IC, MCTX], BF16, tag="kb")     # k raw ints, feat-major
        vb = pp.tile([P, NKB * H * VW], BF16, tag="vb")  # v natural + ones col
        vb3 = vb[:].rearrange("p (k h w) -> p k h w", h=H, w=VW)
        idt = pp.tile([P, P], F32, tag="idt")           # identity for PE transp
        nc.sync.dma_start(out=idt[:], in_=iden.ap()[:, :])
        icT = pp.tile([P, NKB], F32, tag="icT")         # inv_c, ctx-token-major
        vsc = pp.tile([P, NKB], F32, tag="vsc")         # icT * mean|wv|
        esc = pp.tile([P, NKB], F32, tag="esc")         # icT * mq*mk/sqrt(D)

        wmean = {}

        # ---- weight quantization ----------------------------------------
        # ws (fp32 staging) -> Act Abs+accum -> gps allreduce -> qs=1/mean
        # DVE: t=min(w*qs,1.49); t=max(t,-1.49)+MAGIC; tern copy(t-MAGIC)->bf16
        def quant_weight(w, wsp, dst_pool, tern_eng="act"):
            ws = wsp.tile([P, KC * INNER], F32, tag="wst")
            for c in range(KC):
                nc.sync.dma_start(out=ws[:, c * INNER:(c + 1) * INNER],
                                  in_=wT[w].ap()[c * P:(c + 1) * P, :])
            wbt = dst_pool.tile([P, KC * INNER], BF16, tag="wb_" + w,
                                name="wb_" + w)
            wsum = smp.tile([P, 1], F32, tag="wsum")
            nc.scalar.activation(wbt[:], ws[:], AF.Abs, accum_out=wsum[:])
            wrep = smp.tile([P, 1], F32, tag="wrep")
            nc.gpsimd.partition_all_reduce(wrep[:], wsum[:], channels=P,
                                           reduce_op=bass_isa.ReduceOp.add)
            mean = smp.tile([P, 1], F32, tag="wmean_" + w, name="mean_" + w)
            nc.vector.tensor_scalar(mean[:], wrep[:], 1.0 / (DIM * INNER),
                                    EPS, OP.mult, OP.max)
            qs = smp.tile([P, 1], F32, tag="wqs_" + w, name="qs_" + w)
            nc.vector.reciprocal(qs[:], mean[:])
            wmean[w] = mean
            nc.vector.tensor_scalar(ws[:], ws[:], qs[:], 1.49, OP.mult, OP.min)
            nc.vector.tensor_scalar(ws[:], ws[:], -1.49, MAGIC, OP.max, OP.add)
            if tern_eng == "act":
                nc.scalar.activation(wbt[:], ws[:], AF.Copy, bias=-MAGIC)
            else:
                nc.vector.tensor_scalar(wbt[:], ws[:], -MAGIC, None, OP.add)
            return wbt

        # ---- activation quantization (feature-major, no dequant) --------
        # src fp32 [P, KC, ncol]; writes int-valued bf16 into dst[:, c, col0:]
        # and the replicated dequant scale inv_rep[:, 0:ncol].
        def act_quant(asp, src, dst, dcol0, inv_rep, ncol):
            amax = asp.tile([P, ncol], F32, tag="amax")
            nc.vector.tensor_reduce(
                amax[:], src.rearrange("p c t -> p t c"),
                axis=AX.X, op=OP.max, apply_absolute_value=True)
            rep = asp.tile([P, ncol], F32, tag="arep")
            nc.gpsimd.partition_all_reduce(rep[:], amax[:], channels=P,
                                           reduce_op=bass_isa.ReduceOp.max)
            nc.vector.tensor_scalar(inv_rep, rep[:], EPS, 1.0 / 127.0,
                                    OP.max, OP.mult)
            rq = asp.tile([P, ncol], F32, tag="arq")
            nc.vector.reciprocal(rq[:], inv_rep)
            for c in range(KC):
                tmp = asp.tile([P, ncol], F32, tag="atmp")
                nc.vector.tensor_tensor(tmp[:], src[:, c, :], rq[:],
                                        op=OP.mult)
                nc.vector.tensor_scalar(
                    dst[:, c, dcol0:dcol0 + ncol], tmp[:], MAGIC, -MAGIC,
                    OP.add, OP.add)

        ETOK = 256               # ctx eighth size
        NE = MCTX // ETOK        # 8 eighths
        EKB = ETOK // P          # ctx 128-blocks per eighth (2)

        with ExitStack() as ph1:
            asp = ph1.enter_context(tc.tile_pool(name="astage", bufs=1))
            ps_a = ph1.enter_context(tc.tile_pool(name="ps_a", bufs=3,
                                                  space="PSUM"))
            ps_v = ph1.enter_context(tc.tile_pool(name="ps_v", bufs=2,
                                                  space="PSUM"))
            ps_t = ph1.enter_context(tc.tile_pool(name="ps_t", bufs=1,
                                                  space="PSUM"))
            csp = ph1.enter_context(tc.tile_pool(name="cstage", bufs=2))
            cqp = ph1.enter_context(tc.tile_pool(name="cq", bufs=1))
            wsp = ph1.enter_context(tc.tile_pool(name="wstage", bufs=1))

            # quantized ctx in two half tiles (avoids false WAR deps between
            # later eighth writes and earlier K-proj reads)
            cdqh = [cqp.tile([P, KC, MCTX // 2], BF16, tag=f"cdq{h}",
                             name=f"cdq{h}") for h in range(2)]

            def ctx_dma(e):
                cs = csp.tile([P, KC, ETOK], F32, tag="cs", name=f"cs{e}")
                col0 = e * ETOK
                for c in range(KC):
                    nc.sync.dma_start(
                        out=cs[:, c, :],
                        in_=cT.ap()[c * P:(c + 1) * P, col0:col0 + ETOK])
                return cs

            def ctx_quant(e, cs):
                inv_c = csp.tile([P, ETOK], F32, tag="invc", name=f"invc{e}")
                h, lcol = e // (NE // 2), (e % (NE // 2)) * ETOK
                act_quant(asp, cs[:], cdqh[h][:], lcol, inv_c[:], ETOK)
                for kk in range(EKB):
                    kbk = e * EKB + kk
                    pt = ps_t.tile([P, P], F32, tag="pt", name=f"pt{kbk}")
                    nc.tensor.transpose(pt[:], inv_c[:, kk * P:(kk + 1) * P],
                                        idt[:])
                    nc.scalar.copy(icT[:, kbk:kbk + 1], pt[:, 0:1])

            def k_proj(q, wkb3):
                h, lcol = q // 2, (q % 2) * QTOK
                for ic in range(IC):
                    pk = ps_a.tile([P, QTOK], F32, tag="psa",
                                   name=f"pk{q}_{ic}")
                    for c in range(KC):
                        nc.tensor.matmul(
                            pk[:], wkb3[:, c, ic * P:(ic + 1) * P],
                            cdqh[h][:, c, lcol:lcol + QTOK],
                            start=(c == 0), stop=(c == KC - 1))
                    nc.scalar.copy(kb[:, ic, q * QTOK:(q + 1) * QTOK], pk[:])

            # --- front-load x + first two ctx eighths, then compute ------
            with tc.tile_pool(name="xq", bufs=1) as xqp:
                with tc.tile_pool(name="xstage", bufs=1) as xsp:
                    xs = xsp.tile([P, KC, NTOK], F32, tag="xs")
                    for c in range(KC):
                        nc.sync.dma_start(out=xs[:, c, :],
                                          in_=xT.ap()[c * P:(c + 1) * P, :])
                    cs0 = ctx_dma(0)
                    cs1 = ctx_dma(1)
                    xdq = xqp.tile([P, KC, NTOK], BF16, tag="xdq")
                    inv_x = xqp.tile([P, NTOK], F32, tag="invx")
                    act_quant(asp, xs[:], xdq[:], 0, inv_x[:], NTOK)
                ctx_quant(0, cs0)
                ctx_quant(1, cs1)

                # wq + Q projection (eviction scale inv_x)
                with tc.tile_pool(name="wbqq", bufs=1) as wbpq:
                    wqb = quant_weight("wq", wsp, wbpq, tern_eng="act")
                    wqb3 = wqb[:].rearrange("p (c i) -> p c i", c=KC)
                    for ic in range(IC):
                        pq = ps_a.tile([P, NTOK], F32, tag="psa",
                                       name=f"pq{ic}")
                        for c in range(KC):
                            nc.tensor.matmul(
                                pq[:], wqb3[:, c, ic * P:(ic + 1) * P],
                                xdq[:, c, :],
                                start=(c == 0), stop=(c == KC - 1))
                        nc.vector.tensor_tensor(qb[:, ic, :], pq[:],
                                                inv_x[:], op=OP.mult)

            cs2 = ctx_dma(2)
            cs3 = ctx_dma(3)
            ctx_quant(2, cs2)
            ctx_quant(3, cs3)

            with tc.tile_pool(name="wbqk", bufs=1) as wbpk:
                wkb = quant_weight("wk", wsp, wbpk, tern_eng="dve")
                wkb3 = wkb[:].rearrange("p (c i) -> p c i", c=KC)
                k_proj(0, wkb3)
                k_proj(1, wkb3)

                for e in (4, 5, 6, 7):
                    cs = ctx_dma(e)
                    ctx_quant(e, cs)

                with tc.tile_pool(name="wbqv", bufs=1) as wbpv:
                    wvb = quant_weight("wv", wsp, wbpv, tern_eng="dve")
                    wvb3 = wvb[:].rearrange("p (c i) -> p c i", c=KC)
                    k_proj(2, wkb3)
                    k_proj(3, wkb3)

                    # eviction / exp scales (icT complete by now)
                    qkm = smp.tile([P, 1], F32, tag="qkm")
                    nc.vector.tensor_tensor(qkm[:], wmean["wq"][:],
                                            wmean["wk"][:], op=OP.mult)
                    nc.vector.tensor_scalar(qkm[:], qkm[:],
                                            1.0 / float(np.sqrt(D)), None,
                                            OP.mult)
                    nc.vector.tensor_scalar(vsc[:], icT[:], wmean["wv"][:],
                                            None, OP.mult)
                    nc.vector.tensor_scalar(esc[:], icT[:], qkm[:], None,
                                            OP.mult)
                    # ones column of v (evictions only touch [0:D])
                    nc.vector.memset(vb3[:, :, :, D], 1.0)

                    # V projection
                    hph = (INNER // 2) // D  # heads per half (8)
                    for kbk in range(NKB):
                        h, kk = kbk // (NKB // 2), kbk % (NKB // 2)
                        pv = ps_v.tile([P, 2, INNER // 2], F32, tag="psv",
                                       name=f"pv{kbk}")
                        for c in range(KC):
                            for ih in range(2):
                                nc.tensor.matmul(
                                    pv[:, ih, :],
                                    cdqh[h][:, c, kk * P:(kk + 1) * P],
                                    wvb3[:, c, ih * (INNER // 2):
                                         (ih + 1) * (INNER // 2)],
                                    start=(c == 0), stop=(c == KC - 1))
                        for ih in range(2):
                            nc.scalar.mul(
                                vb3[:, kbk, ih * hph:(ih + 1) * hph, 0:D],
                                pv[:, ih, :].rearrange(
                                    "p (h d) -> p h d", d=D),
                                vsc[:, kbk:kbk + 1])

        # ---- attention ---------------------------------------------------
        wop = ctx.enter_context(tc.tile_pool(name="wopool", bufs=1))
        op_pool = ctx.enter_context(tc.tile_pool(name="opool", bufs=1))
        otT = op_pool.tile([P, IC, NTOK], F32, tag="otT")
        with ExitStack() as ph2:
            ep = ph2.enter_context(tc.tile_pool(name="etile", bufs=4))
            rbp = ph2.enter_context(tc.tile_pool(name="rbpool", bufs=2))
            ps_sc = ph2.enter_context(tc.tile_pool(name="ps_sc", bufs=2,
                                                   space="PSUM"))
            ps_o = ph2.enter_context(tc.tile_pool(name="ps_o", bufs=2,
                                                  space="PSUM"))
            ps_o2 = ph2.enter_context(tc.tile_pool(name="ps_o2", bufs=2,
                                                   space="PSUM"))
            wsp2 = ph2.enter_context(tc.tile_pool(name="wstage2", bufs=1))

            wob = None
            for hp in range(H // 2):
                hA, hB = 2 * hp, 2 * hp + 1
                pA, pB = (hA * D) % P, (hB * D) % P
                popool, potag = (ps_o, "po") if hp % 2 == 0 else (ps_o2, "po2")
                po = [popool.tile([VW, NTOK], F32, tag=potag,
                                  name=f"po{hp}_{j}") for j in range(2)]
                for kbk in range(NKB):
                    ss = ps_sc.tile([P, 2, NTOK], F32, tag="ss", name="ss")
                    for j, (h, ph) in enumerate([(hA, pA), (hB, pB)]):
                        nc.tensor.matmul(
                            ss[:, j, :],
                            kb[ph:ph + D, hp, kbk * P:(kbk + 1) * P],
                            qb[ph:ph + D, hp, :],
                            start=True, stop=True)
                    et = ep.tile([P, 2, NTOK], BF16, tag="et")
                    nc.scalar.activation(et[:], ss[:], AF.Exp,
                                         scale=esc[:, kbk:kbk + 1])
                    for j, h in enumerate((hA, hB)):
                        nc.tensor.matmul(
                            po[j][0:VW, :],
                            vb3[:, kbk, h, :],
                            et[:, j, :],
                            start=(kbk == 0), stop=(kbk == NKB - 1))
                # normalize: ot = po[0:D] / den  (den = po[D] row)
                for j, ph in enumerate((pA, pB)):
                    rd = rbp.tile([1, NTOK], F32, tag="rd")
                    nc.vector.reciprocal(rd[:], po[j][D:D + 1, :])
                    rb = rbp.tile([D, NTOK], F32, tag="rb")
                    nc.gpsimd.partition_broadcast(rb[:], rd[:])
                    nc.vector.tensor_tensor(
                        otT[ph:ph + D, hp, :], po[j][0:D, :], rb[:],
                        op=OP.mult)
                if hp == 0:
                    # wo quant rides the attention phase (engines are free)
                    wob = quant_weight("wo", wsp2, wop, tern_eng="dve")
                if hp == 3:
                    # first half of the out-quant absmax (runs mid-attention)
                    oam1 = op_pool.tile([P, NTOK], F32, tag="oam1")
                    nc.vector.tensor_reduce(
                        oam1[:], otT[:, 0:4, :].rearrange("p c t -> p t c"),
                        axis=AX.X, op=OP.max, apply_absolute_value=True)

        # ---- attn-out quantization + output projection ------------------
        with tc.tile_pool(name="oq", bufs=2) as oqp, \
                tc.tile_pool(name="ysb", bufs=2) as yp, \
                tc.tile_pool(name="ps_y", bufs=2, space="PSUM") as ps_y, \
                tc.tile_pool(name="ps_t2", bufs=1, space="PSUM") as ps_t2:
            odq = op_pool.tile([P, IC, NTOK], BF16, tag="odq")
            inv_o = op_pool.tile([P, NTOK], F32, tag="invo")
            # out act-quant (absmax split: first half computed mid-attention)
            oam = oqp.tile([P, NTOK], F32, tag="oam")
            nc.vector.tensor_reduce(
                oam[:], otT[:, 4:8, :].rearrange("p c t -> p t c"),
                axis=AX.X, op=OP.max, apply_absolute_value=True)
            nc.vector.tensor_tensor(oam[:], oam[:], oam1[:], op=OP.max)
            orep = oqp.tile([P, NTOK], F32, tag="orep")
            nc.gpsimd.partition_all_reduce(orep[:], oam[:], channels=P,
                                           reduce_op=bass_isa.ReduceOp.max)
            nc.vector.tensor_scalar(inv_o[:], orep[:], EPS, 1.0 / 127.0,
                                    OP.max, OP.mult)
            orq = oqp.tile([P, NTOK], F32, tag="orq")
            nc.vector.reciprocal(orq[:], inv_o[:])
            for c in range(KC):
                otmp = oqp.tile([P, NTOK], F32, tag="otmp")
                nc.vector.tensor_tensor(otmp[:], otT[:, c, :], orq[:],
                                        op=OP.mult)
                nc.vector.tensor_scalar(odq[:, c, :], otmp[:], MAGIC, -MAGIC,
                                        OP.add, OP.add)

            # y-eviction scale, token-major: syT = (inv_o).T * mean|wo|
            syT = smp.tile([P, NTB], F32, tag="syT")
            for tb in range(NTB):
                pt = ps_t2.tile([P, P], F32, tag="pt2", name=f"pt2{tb}")
                nc.tensor.transpose(pt[:], inv_o[:, tb * P:(tb + 1) * P],
                                    idt[:])
                nc.scalar.copy(syT[:, tb:tb + 1], pt[:, 0:1])
            nc.vector.tensor_scalar(syT[:], syT[:], wmean["wo"][:], None,
                                    OP.mult)

            wob3 = wob[:].rearrange("p (c i) -> p c i", c=IC)
            for tb in range(NTB):
                py = ps_y.tile([P, 2, DIM // 2], F32, tag="psy",
                               name=f"py{tb}")
                for c in range(IC):
                    for oh in range(2):
                        nc.tensor.matmul(
                            py[:, oh, :],
                            odq[:, c, tb * P:(tb + 1) * P],
                            wob3[:, c, oh * (DIM // 2):(oh + 1) * (DIM // 2)],
                            start=(c == 0), stop=(c == IC - 1))
                ysb = yp.tile([P, DIM], F32, tag="ysb")
                nc.scalar.mul(ysb[:], py[:].rearrange("p a b -> p (a b)"),
                              syT[:, tb:tb + 1])
                nc.sync.dma_start(
                    out=y_out.ap()[tb * P:(tb + 1) * P, :], in_=ysb[:])
    nc.compile()
    return nc


_CACHE = {}


def _get_nc(key, cfg):
    if key not in _CACHE:
        _CACHE[key] = build(cfg)
    return _CACHE[key]


def _shard(x, context, wq, wk, wv, wo, NTOK):
    b = x.shape[0]
    wmaps = {w + "T": np.ascontiguousarray(a.T)
             for w, a in (("wq", wq), ("wk", wk), ("wv", wv), ("wo", wo))}
    wmaps["iden"] = np.eye(128, dtype=np.float32)
    cores_per_b = N_CORES // b
    in_maps = []
    for core in range(N_CORES):
        bi = core // cores_per_b
        t0 = (core % cores_per_b) * NTOK
        in_maps.append(dict(
            xT=np.ascontiguousarray(x[bi, t0:t0 + NTOK, :].T),
            cT=np.ascontiguousarray(context[bi].T),
            **wmaps))
    return in_maps


def _assemble(results, b, n, dim, NTOK):
    out = np.empty((b, n, dim), dtype=np.float32)
    cores_per_b = N_CORES // b
    for core in range(N_CORES):
        bi = core // cores_per_b
        t0 = (core % cores_per_b) * NTOK
        out[bi, t0:t0 + NTOK, :] = results[core]["y"]
    return out


def run(x, context, wq, wk, wv, wo, trace=False):
    cfg = CFG_FULL
    b, n, dim = x.shape
    NTOK = cfg["NTOK"]
    nc = _get_nc("full", cfg)
    in_maps = _shard(x, context, wq, wk, wv, wo, NTOK)
    res = run_bass_kernel_spmd(nc, in_maps, list(range(N_CORES)), trace=trace)
    return _assemble(res.results, b, n, dim, NTOK), res


def kernel(x, context, wq, wk, wv, wo):
    return run(x, context, wq, wk, wv, wo, trace=False)[0]


if __name__ == "__main__":
    ins = {k: np.random.randn(*s).astype(np.float32) * (0.02 if k[0] == 'w' else 1.0)
           for k, s in [("x", (2, 2048, 1024)), ("context", (2, 2048, 1024)),
                        ("wq", (1024, 1024)), ("wk", (1024, 1024)),
                        ("wv", (1024, 1024)), ("wo", (1024, 1024))]}
    y = kernel(**ins)
    print("kernel output", y.shape, y.dtype, np.abs(y).max())
